# revision 68
# baseline (speedup 1.0000x reference)
"""Causal self-attention with LoRA (folded host-side), sharded over 8 NeuronCores.

Sharding: core c -> batch b = c//4, head-group g = c%4 (4 heads of 16).
Each core computes out[b, :, 256g:256g+256]; no collectives needed.

All matmuls run in bf16 (fp32 PSUM accumulation); host supplies x already
transposed and bf16-cast, so the device does no transposes at all:

  xT    [128p, kd(8), t]            DMA'd directly (host provides x^T bf16)
  Q^T/K^T [o(128p), ot(2), t] bf16  proj matmuls, lhsT=W^T tile, rhs=xT
  V_aug [s(128p), tt, h(4), 65]     proj matmuls, lhsT=xT tile, rhs=W^T;
                                    col 64 = ones (softmax denominator row)
  scores^T [s(128p), t-chunk] psum = K^T_h.T @ Q^T_h  (64-part contraction,
                                    head pairs at base partitions 0/64)
  E = exp(scores*0.125 + mask[s])   ACT -> bf16; causal: skip s-tiles above
                                    diag, memset sub-block zeros, gpsimd
                                    affine_select on diagonal blocks
  numer [65, qb(4), 256] psum      += V_aug_h.T @ E per 256-col block; row 64
                                    = Z[t]; DMA'd PSUM->DRAM when the block's
                                    accumulation stops.  Host does the final
                                    transpose + divide by Z.
"""

import numpy as np
from contextlib import ExitStack

import concourse.bass as bass
import concourse.tile as tile
from concourse import bacc, mybir
from concourse.bass_utils import run_bass_kernel_spmd

B, T_FULL, DM, H, R = 2, 2048, 1024, 16, 8
HD = 64
NHC = 4            # heads per core
OC = NHC * HD      # 256 out cols per core
LORA_SCALE = 16.0 / R
F32 = mybir.dt.float32
BF16 = mybir.dt.bfloat16
F8 = mybir.dt.float8e4
AF = mybir.ActivationFunctionType
ALU = mybir.AluOpType
DR = mybir.MatmulPerfMode.DoubleRow
P = 128
SCORES_FP8 = True  # QK^T in fp8e4m3 DoubleRow (zero-padded second k-slot)


def build_program(T=T_FULL, PACKED=True):
    KD = DM // P              # 8 contraction tiles
    NTT = T // P              # t 128-tiles
    CH = min(1024, T)         # attention t-chunk
    NJ = T // CH
    NQB = CH // 256           # 256-wide PV blocks per chunk

    nc = bacc.Bacc("TRN2", target_bir_lowering=False, debug=False)
    xt_d = nc.dram_tensor("xt", [DM, T], BF16, kind="ExternalInput").ap()
    wqt_d = nc.dram_tensor("wqt", [P, KD, OC], BF16, kind="ExternalInput").ap()
    wkt_d = nc.dram_tensor("wkt", [P, KD, OC], BF16, kind="ExternalInput").ap()
    wvt_d = nc.dram_tensor("wvt", [P, KD, OC], BF16, kind="ExternalInput").ap()
    bq_d = nc.dram_tensor("bq2", [P, 2], F32, kind="ExternalInput").ap()
    bk_d = nc.dram_tensor("bk2", [P, 2], F32, kind="ExternalInput").ap()
    mask_d = nc.dram_tensor("mask", [P, NTT], F32, kind="ExternalInput").ap()
    # ident: identity; trim: [0:128]=-big, [128:256]=causal triangle
    # (-big where p > col-128), [256:384]=0.  Used to bias masked score
    # regions inside the scores PSUM accumulation group (no Pool dependency).
    # fp8 variants carry a DoubleRow zero k-slot and -240 (e4m3 max is 448).
    id_d = nc.dram_tensor("ident", [P, P], BF16, kind="ExternalInput").ap()
    trim_d = nc.dram_tensor("trim", [P, 384], BF16, kind="ExternalInput").ap()
    id8_d = nc.dram_tensor("ident8", [P, 2, P], F8, kind="ExternalInput").ap()
    trim8_d = nc.dram_tensor("trim8", [P, 2, 384], F8, kind="ExternalInput").ap()
    # numer[h, d(64)+Z(1), tb, 256]: unnormalized attention + denominator row
    num_d = nc.dram_tensor("num", [NHC, HD + 1, T // 256, 256], F32,
                           kind="ExternalOutput").ap()

    with tile.TileContext(nc) as tc, ExitStack() as ctx:
        const = ctx.enter_context(tc.tile_pool(name="const", bufs=1))
        wpool = ctx.enter_context(tc.tile_pool(name="w", bufs=1))
        big = ctx.enter_context(tc.tile_pool(name="big", bufs=1))
        epool = ctx.enter_context(tc.tile_pool(name="e", bufs=8))
        opool = ctx.enter_context(tc.tile_pool(name="osb", bufs=4))
        ps_sc = ctx.enter_context(tc.tile_pool(name="ps_sc", bufs=3, space="PSUM"))
        ps_pv = ctx.enter_context(tc.tile_pool(name="ps_pv", bufs=1, space="PSUM"))

        dq = [nc.sync, nc.scalar, nc.gpsimd]

        # ---- load order tuned for ramp: wq/wk, then the x^T quarters the
        # first projection chunk needs, then wv/consts, then the rest ----
        wq_sb = wpool.tile([P, KD, OC], BF16, tag="wq")
        nc.sync.dma_start(wq_sb[:], wqt_d[:])
        wk_sb = wpool.tile([P, KD, OC], BF16, tag="wk")
        nc.gpsimd.dma_start(wk_sb[:], wkt_d[:])

        xT = big.tile([P, KD, T], BF16, tag="xT")

        def load_xt(kd, t0, t1, qi):
            dq[qi % 3].dma_start(
                xT[:, kd, t0:t1], xt_d[kd * P:(kd + 1) * P, t0:t1]
            )

        LW = min(512, T)
        for kd in range(KD):
            load_xt(kd, 0, LW, kd)
        if T > LW:
            # second quarter next: the projection prefix pieces for chunk 0
            # block the in-order PE until these land
            for kd in range(KD):
                load_xt(kd, LW, 2 * LW, kd + 1)
        wv_sb = wpool.tile([P, KD, OC], BF16, tag="wv")
        nc.scalar.dma_start(wv_sb[:], wvt_d[:])

        bq_sb = const.tile([P, 2], F32)
        nc.sync.dma_start(bq_sb[:], bq_d[:])
        bk_sb = const.tile([P, 2], F32)
        nc.gpsimd.dma_start(bk_sb[:], bk_d[:])
        mask_sb = const.tile([P, NTT], F32)
        nc.sync.dma_start(mask_sb[:], mask_d[:])
        if SCORES_FP8:
            id8_sb = const.tile([P, 2, P], F8)
            nc.scalar.dma_start(id8_sb[:], id8_d[:])
            trim8_sb = const.tile([P, 2, 384], F8)
            nc.scalar.dma_start(trim8_sb[:], trim8_d[:])
        else:
            id_sb = const.tile([P, P], BF16)
            nc.scalar.dma_start(id_sb[:], id_d[:])
            trim_sb = const.tile([P, 384], BF16)
            nc.scalar.dma_start(trim_sb[:], trim_d[:])
        for half in range(2, T // LW):
            for kd in range(KD):
                load_xt(kd, half * LW, (half + 1) * LW, kd + half)

        QK_DT = F8 if SCORES_FP8 else BF16
        # dims: [partition, o-tile, DoubleRow k-slot (1 = zeros), t]
        QT = big.tile([P, 2, 2, T], QK_DT, tag="QT")
        KT = big.tile([P, 2, 2, T], QK_DT, tag="KT")
        if SCORES_FP8:
            nc.gpsimd.memset(QT[:, :, 1, :], 0.0)
            nc.gpsimd.memset(KT[:, :, 1, :], 0.0)
        V = big.tile([P, NTT, NHC, HD + 1], BF16, tag="V")
        ones_sb = const.tile([P, 1], F32)
        nc.gpsimd.memset(ones_sb[:], 1.0)
        nc.vector.tensor_copy(
            V[:, :, :, HD:HD + 1].rearrange("p a b c -> p (a b c)"),
            ones_sb[:, 0:1].to_broadcast((P, NTT * NHC)),
        )

        def front_pieces(jp):
            """Front-end work for chunk jp as small thunks, interleaved into
            attention i-loops.  PSUM comes from the shared "sc" tag (one
            accumulation group per piece, bank-sequential)."""
            t0c = jp * CH

            def qk_piece(dst, w_sb, b_sb, ot, c0p, w):
                def go():
                    pr = ps_sc.tile([P, CH], F32, tag="sc",
                                    name=f"prj_{jp}_{c0p}_{ot}_{id(dst) % 97}")
                    for kd in range(KD):
                        nc.tensor.matmul(
                            pr[:, :w],
                            lhsT=w_sb[:, kd, ot * P:(ot + 1) * P],
                            rhs=xT[:, kd, t0c + c0p:t0c + c0p + w],
                            start=(kd == 0),
                            stop=(kd == KD - 1),
                        )
                    nc.vector.tensor_scalar_add(
                        dst[:, ot, 0, t0c + c0p:t0c + c0p + w],
                        pr[:, :w],
                        b_sb[:, ot:ot + 1],
                    )
                return go

            def v_piece(tt):
                def go():
                    pr = ps_sc.tile([P, CH], F32, tag="sc", name=f"prv_{tt}")
                    for kd in range(KD):
                        nc.tensor.matmul(
                            pr[:, :OC],
                            lhsT=xT[:, kd, tt * P:(tt + 1) * P],
                            rhs=wv_sb[:, kd, :],
                            start=(kd == 0),
                            stop=(kd == KD - 1),
                        )
                    nc.vector.tensor_copy(
                        V[:, tt, :, 0:HD],
                        pr[:, :OC].rearrange("p (h d) -> p h d", h=NHC)
                    )
                return go

            qs, ks, vs = [], [], []
            for c0p in range(0, CH, 512):
                w = min(512, CH - c0p)
                for ot in range(2):
                    qs.append(qk_piece(QT, wq_sb, bq_sb, ot, c0p, w))
                for ot in range(2):
                    ks.append(qk_piece(KT, wk_sb, bk_sb, ot, c0p, w))
                for tt in range((t0c + c0p) // P, (t0c + c0p + w) // P):
                    vs.append(v_piece(tt))
            return qs, ks, vs

        # ---- attention.  PV for step i is emitted LAG steps behind scores
        # so the in-order PE never waits on exp(i): while ACT computes
        # exp(i), the PE streams scores(i+1), scores(i+2).  With
        # deferred=<list>, PV is not emitted inline at all: thunks are
        # appended for later phases (pure-PE filler for exp-bound stretches);
        # the head's E tiles persist until those thunks consume them. ----
        def head_stream(j, h, deferred=None):
            ho, hb = divmod(h, 2)
            hbase = hb * 64
            imax = (CH * (j + 1) - 1) // P
            box = {}

            def get_pv():
                if "pv" not in box:
                    box["pv"] = ps_pv.tile([HD + 1, NQB, 256], F32, tag="pv",
                                           name=f"pv_{j}_{h}")
                    box["osb"] = opool.tile([HD + 1, CH], F32, tag="osb",
                                            name=f"osb_{j}_{h}")
                return box["pv"], box["osb"]

            Es = {}
            etag = "E0" if deferred is not None else "E"
            ebufs = 33 if deferred is not None else 8

            def kq_mm(ps, p0, s0, w, start, stop, i):
                if SCORES_FP8:
                    nc.tensor.matmul(
                        ps[:, p0:p0 + w],
                        lhsT=KT[hbase:hbase + 64, ho, :, i * P:(i + 1) * P],
                        rhs=QT[hbase:hbase + 64, ho, :,
                               j * CH + s0:j * CH + s0 + w],
                        start=start, stop=stop, perf_mode=DR,
                    )
                else:
                    nc.tensor.matmul(
                        ps[:, p0:p0 + w],
                        lhsT=KT[hbase:hbase + 64, ho, 0, i * P:(i + 1) * P],
                        rhs=QT[hbase:hbase + 64, ho, 0,
                               j * CH + s0:j * CH + s0 + w],
                        start=start, stop=stop,
                    )

            def emit_scores(g, gi):
                """Score+exp for a group of s-tiles.  Packed (len>1) groups
                share one PSUM tile and ONE exp; member data sits at
                anchor_a + col (anchored at its 256-block start, so every
                straddle hole is tri-blanked and exp-ed to ~0)."""
                ps = ps_sc.tile([P, CH], F32, tag="sc",
                                name=f"sc_{j}_{h}_{gi}")
                E = epool.tile([P, CH], BF16, tag=etag, bufs=ebufs,
                               name=f"E_{j}_{h}_{gi}")
                anchors = {}
                base = 0
                packed = len(g) > 1
                for a in g:
                    c0, diag, s0b = tile_geo(j, a)
                    anchor = (base - s0b) if packed else 0
                    anchors[a] = anchor
                    if diag:
                        # diag-containing 256-block: tri bias opens the
                        # group (-1e8/-240 on the masked region), KQ closes
                        off = P - (c0 - s0b)
                        w = min(256, CH - s0b)
                        if SCORES_FP8:
                            nc.tensor.matmul(
                                ps[:, anchor + s0b:anchor + s0b + w],
                                lhsT=id8_sb[:],
                                rhs=trim8_sb[:, :, off:off + w],
                                start=True, stop=False, perf_mode=DR,
                            )
                        else:
                            nc.tensor.matmul(
                                ps[:, anchor + s0b:anchor + s0b + w],
                                lhsT=id_sb[:],
                                rhs=trim_sb[:, off:off + w],
                                start=True, stop=False,
                            )
                        kq_mm(ps, anchor + s0b, s0b, w, False, True, a)
                        s0 = s0b + w
                    else:
                        s0 = s0b
                    while s0 < CH:
                        p0 = anchor + s0
                        w = min(512 if p0 % 512 == 0 else 256, CH - s0)
                        kq_mm(ps, p0, s0, w, True, True, a)
                        s0 += w
                    base += CH - s0b
                Es[gi] = (E, anchors)
                if packed:
                    nc.scalar.activation(
                        E[:, 0:base], ps[:, 0:base], AF.Exp,
                        scale=float(HD) ** -0.5,
                        bias=mask_sb[:, g[0]:g[0] + 1],
                    )
                else:
                    a = g[0]
                    c0, diag, s0b = tile_geo(j, a)
                    if c0 > s0b:
                        nc.gpsimd.memset(E[:, s0b:c0], 0.0)
                    nc.scalar.activation(
                        E[:, c0:CH], ps[:, c0:CH], AF.Exp,
                        scale=float(HD) ** -0.5, bias=mask_sb[:, a:a + 1],
                    )

            def emit_pv(g, gi):
                E, anchors = Es.pop(gi)
                for a in g:
                    emit_pv_one(a, E, anchors[a])

            def emit_pv_one(i, E, anchor):
                pv, osb = get_pv()
                c0, diag, s0b = tile_geo(j, i)
                if i == 0:
                    # one start=True matmul per PSUM BANK: a later start in
                    # the same bank wipes an open group's partial sum, so
                    # banks must be opened whole (2 qb blocks = 1 bank).
                    for qb0 in range(0, NQB, 2):
                        nb = min(2, NQB - qb0)
                        nc.tensor.matmul(
                            pv[:, qb0:qb0 + nb, :].rearrange(
                                "p a b -> p (a b)"),
                            lhsT=V[:, i, h, :],
                            rhs=E[:, anchor + qb0 * 256:
                                   anchor + (qb0 + nb) * 256],
                            start=True,
                            stop=False,
                        )
                    return
                qb = 0
                while qb < NQB:
                    s0 = qb * 256
                    if s0 + 256 <= s0b:
                        qb += 1
                        continue
                    lasti = min(imax, (256 * (qb + 1) + CH * j) // P - 1)
                    if (qb % 2 == 0 and qb + 1 < NQB and s0 >= s0b
                            and i < lasti):
                        # both blocks of this bank continue: one 512-wide
                        # accumulate
                        nc.tensor.matmul(
                            pv[:, qb:qb + 2, :].rearrange("p a b -> p (a b)"),
                            lhsT=V[:, i, h, :],
                            rhs=E[:, anchor + s0:anchor + s0 + 512],
                            start=False,
                            stop=False,
                            skip_group_check=True,
                        )
                        qb += 2
                        continue
                    nc.tensor.matmul(
                        pv[:, qb, :],
                        lhsT=V[:, i, h, :],
                        rhs=E[:, anchor + s0:anchor + s0 + 256],
                        start=False,
                        stop=(i == lasti),
                        skip_group_check=True,
                    )
                    if i == lasti:
                        nc.vector.tensor_copy(
                            osb[:, qb * 256:(qb + 1) * 256], pv[:, qb, :]
                        )
                        last_head = (j == NJ - 1 and h == NHC - 1)
                        if last_head or qb == NQB - 1:
                            b0 = qb if last_head else 0
                            tb0 = (CH * j) // 256
                            dq[(2 * h + qb) % 3].dma_start(
                                num_d[h, :, tb0 + b0:tb0 + qb + 1, :],
                                osb[:, b0 * 256:(qb + 1) * 256].rearrange(
                                    "p (tb ti) -> p tb ti", ti=256),
                            )
                    qb += 1

            plan = head_plan(j)
            ng = len(plan)
            if deferred is None:
                for k in range(ng + LAG):
                    if 0 <= k - LAG < ng:
                        emit_pv(plan[k - LAG], k - LAG)
                    if k < ng:
                        emit_scores(plan[k], k)
                    yield
            else:
                for k in range(ng):
                    emit_scores(plan[k], k)
                    yield
                for k in range(ng):
                    deferred.append(lambda k=k: emit_pv(plan[k], k))

        def tile_geo(j, i):
            s0_abs = i * P
            t0_abs = CH * j
            c0 = 0 if s0_abs < t0_abs else s0_abs - t0_abs
            diag = s0_abs >= t0_abs
            s0b = c0 - c0 % 256
            return c0, diag, s0b

        def head_plan(j):
            """s-tile processing groups: adjacent narrow diagonal tiles are
            packed into one scores-psum/exp when their block-anchored widths
            fit a single CH-wide tile (requires a uniform attention mask)."""
            imax = (CH * (j + 1) - 1) // P
            plan = []
            i = 0
            while i <= imax:
                c0, diag, s0b = tile_geo(j, i)
                if PACKED and diag and i + 1 <= imax:
                    _, _, s0b1 = tile_geo(j, i + 1)
                    if (CH - s0b) + (CH - s0b1) <= CH:
                        plan.append((i, i + 1))
                        i += 2
                        continue
                plan.append((i,))
                i += 1
            return plan

        LAG = 3

        def n_steps(j, deferred=False):
            return len(head_plan(j)) + (0 if deferred else LAG)

        def run_streams(streams, pending):
            """Proportionally interleave step streams; pop one filler piece
            after each step (more if a backlog would remain)."""
            keyed = []
            for si, (gen, ln) in enumerate(streams):
                for k in range(ln):
                    keyed.append(((k + 0.5) / ln, si, gen))
            keyed.sort(key=lambda t: (t[0], t[1]))
            nleft = len(keyed)
            for _, _, gen in keyed:
                next(gen, None)
                nleft -= 1
                npop = 1 if pending else 0
                if len(pending) > nleft:
                    npop = max(npop, len(pending) - nleft)
                for _ in range(npop):
                    pending.pop(0)()
            for _, (gen, _ln) in enumerate(streams):
                for _ in gen:
                    pass

        f0q, f0k, f0v = front_pieces(0)
        if NJ == 1:
            for piece in f0q + f0k + f0v:
                piece()
            for h in range(NHC):
                run_streams([(head_stream(0, h), n_steps(0))], [])
        else:
            # Minimal serial prefix (what head (0,0) needs up front), then
            # every remaining projection piece dribbles into attention steps
            # as late as its first consumer allows, to fill the exp-bound
            # stretches with PE work.
            f1q, f1k, f1v = front_pieces(1)
            for piece in f0q:
                piece()
            f0k[0]()
            n0, n1 = n_steps(0), n_steps(1)
            run_streams([(head_stream(0, 0), n0)],
                        [f0v[0], f0v[1], f0k[2]] + f0v[2:8] + [f0k[1]])
            run_streams([(head_stream(0, 1), n0)], [f0k[3]] + f1q)
            run_streams([(head_stream(0, 2), n0)], [f1k[0], f1k[2]])
            run_streams([(head_stream(0, 3), n0)], [f1k[1], f1k[3]])
            run_streams([(head_stream(1, 0), n1)], list(f1v))
            for h in range(1, NHC):
                run_streams([(head_stream(1, h), n1)], [])

    nc.compile()
    return nc


def make_in_maps(hidden_states, attention_mask, Wq, bq, Aq, Bq, Wk, bk,
                 Wv, bv, Av, Bv):
    import ml_dtypes
    bf = ml_dtypes.bfloat16
    f32 = np.float32
    weff_q = np.asarray(Wq, f32) + f32(LORA_SCALE) * (
        np.asarray(Bq, f32) @ np.asarray(Aq, f32)
    )
    weff_v = np.asarray(Wv, f32) + f32(LORA_SCALE) * (
        np.asarray(Bv, f32) @ np.asarray(Av, f32)
    )
    Wk = np.asarray(Wk, f32)
    hs = np.asarray(hidden_states, f32)
    am = np.asarray(attention_mask, f32)
    bq = np.asarray(bq, f32)
    bk = np.asarray(bk, f32)
    T = hs.shape[1]
    KD = DM // P

    def warr(w_rows):
        # [OC, DM] row-slice -> lhsT layout [128, KD, OC] bf16
        return np.ascontiguousarray(
            w_rows.T.reshape(KD, P, OC).transpose(1, 0, 2).astype(bf)
        )

    ident = np.ascontiguousarray(np.eye(P, dtype=np.float32).astype(bf))
    trim = np.zeros((P, 384), np.float32)
    trim[:, 0:P] = -1e8
    for q in range(P):
        trim[q + 1:, P + q] = -1e8
    trimf = trim
    trim = np.ascontiguousarray(trim.astype(bf))
    f8 = ml_dtypes.float8_e4m3
    ident8 = np.zeros((P, 2, P), np.float32)
    ident8[:, 0, :] = np.eye(P, dtype=np.float32)
    ident8 = np.ascontiguousarray(ident8.astype(f8))
    trim8 = np.zeros((P, 2, 384), np.float32)
    trim8[:, 0, :] = np.where(trimf < 0, -240.0, 0.0)
    trim8 = np.ascontiguousarray(trim8.astype(f8))

    in_maps = []
    xt_by_batch = {}
    for c in range(8):
        b, g = divmod(c, 4)
        rows = slice(g * OC, (g + 1) * OC)
        if b not in xt_by_batch:
            xt_by_batch[b] = np.ascontiguousarray(hs[b].T.astype(bf))
        in_maps.append({
            "xt": xt_by_batch[b],
            "wqt": warr(weff_q[rows]),
            "wkt": warr(Wk[rows]),
            "wvt": warr(weff_v[rows]),
            "bq2": np.ascontiguousarray(bq[rows].reshape(2, P).T),
            "bk2": np.ascontiguousarray(bk[rows].reshape(2, P).T),
            "mask": np.ascontiguousarray(am[b, 0, 0].reshape(T // P, P).T),
            "ident": ident,
            "trim": trim,
            "ident8": ident8,
            "trim8": trim8,
        })
    return in_maps


_NC_CACHE = {}


def kernel(hidden_states, attention_mask, Wq, bq, Aq, Bq, Wk, bk, Wv, bv,
           Av, Bv, _trace=False):
    T = np.asarray(hidden_states).shape[1]
    am = np.asarray(attention_mask)
    # packed exp groups share one per-partition bias column; only valid
    # when the attention mask is uniform
    packed = bool(np.all(am == am.flat[0]))
    key = (T, packed)
    if key not in _NC_CACHE:
        _NC_CACHE[key] = build_program(T, PACKED=packed)
    nc = _NC_CACHE[key]
    in_maps = make_in_maps(hidden_states, attention_mask, Wq, bq, Aq, Bq,
                           Wk, bk, Wv, bv, Av, Bv)
    res = None
    for attempt in range(3):
        try:
            res = run_bass_kernel_spmd(nc, in_maps, list(range(8)), trace=_trace)
            break
        except Exception:
            # transient NRT_EXEC_UNIT_UNRECOVERABLE device wedges recover on retry
            if attempt == 2:
                raise
            import time as _time
            _time.sleep(15)
    bv = np.asarray(bv, np.float32)
    out = np.empty((B, T, DM), np.float32)
    for c in range(8):
        b, g = divmod(c, 4)
        cols = slice(g * OC, (g + 1) * OC)
        num = res.results[c]["num"].reshape(NHC, HD + 1, T)  # [h, d|Z, t]
        o = num[:, :HD, :] / num[:, HD:HD + 1, :]            # [h, d, t]
        out[b, :, cols] = o.transpose(2, 0, 1).reshape(T, OC) + bv[cols][None, :]
    kernel.last_result = res
    return out


# revision 73
# speedup vs baseline: 1.0006x; 1.0006x over previous
"""Causal self-attention with LoRA (folded host-side), sharded over 8 NeuronCores.

Sharding: core c -> batch b = c//4, head-group g = c%4 (4 heads of 16).
Each core computes out[b, :, 256g:256g+256]; no collectives needed.

All matmuls run in bf16 (fp32 PSUM accumulation); host supplies x already
transposed and bf16-cast, so the device does no transposes at all:

  xT    [128p, kd(8), t]            DMA'd directly (host provides x^T bf16)
  Q^T/K^T [o(128p), ot(2), t] bf16  proj matmuls, lhsT=W^T tile, rhs=xT
  V_aug [s(128p), tt, h(4), 65]     proj matmuls, lhsT=xT tile, rhs=W^T;
                                    col 64 = ones (softmax denominator row)
  scores^T [s(128p), t-chunk] psum = K^T_h.T @ Q^T_h  (64-part contraction,
                                    head pairs at base partitions 0/64)
  E = exp(scores*0.125 + mask[s])   ACT -> bf16; causal: skip s-tiles above
                                    diag, memset sub-block zeros, gpsimd
                                    affine_select on diagonal blocks
  numer [65, qb(4), 256] psum      += V_aug_h.T @ E per 256-col block; row 64
                                    = Z[t]; DMA'd PSUM->DRAM when the block's
                                    accumulation stops.  Host does the final
                                    transpose + divide by Z.
"""

import numpy as np
from contextlib import ExitStack

import concourse.bass as bass
import concourse.tile as tile
from concourse import bacc, mybir
from concourse.bass_utils import run_bass_kernel_spmd

B, T_FULL, DM, H, R = 2, 2048, 1024, 16, 8
HD = 64
NHC = 4            # heads per core
OC = NHC * HD      # 256 out cols per core
LORA_SCALE = 16.0 / R
F32 = mybir.dt.float32
BF16 = mybir.dt.bfloat16
F8 = mybir.dt.float8e4
AF = mybir.ActivationFunctionType
ALU = mybir.AluOpType
DR = mybir.MatmulPerfMode.DoubleRow
P = 128
SCORES_FP8 = True  # QK^T in fp8e4m3 DoubleRow (zero-padded second k-slot)


def build_program(T=T_FULL, PACKED=True):
    KD = DM // P              # 8 contraction tiles
    NTT = T // P              # t 128-tiles
    CH = min(1024, T)         # attention t-chunk
    NJ = T // CH
    NQB = CH // 256           # 256-wide PV blocks per chunk

    nc = bacc.Bacc("TRN2", target_bir_lowering=False, debug=False)
    xt_d = nc.dram_tensor("xt", [DM, T], BF16, kind="ExternalInput").ap()
    wqt_d = nc.dram_tensor("wqt", [P, KD, OC], BF16, kind="ExternalInput").ap()
    wkt_d = nc.dram_tensor("wkt", [P, KD, OC], BF16, kind="ExternalInput").ap()
    wvt_d = nc.dram_tensor("wvt", [P, KD, OC], BF16, kind="ExternalInput").ap()
    # packed f32 consts: [0:2]=bq2, [2:4]=bk2, [4:4+NTT]=mask
    cst_d = nc.dram_tensor("cst", [P, 4 + NTT], F32, kind="ExternalInput").ap()
    # ident: identity; trim: [0:128]=-big, [128:256]=causal triangle
    # (-big where p > col-128), [256:384]=0.  Used to bias masked score
    # regions inside the scores PSUM accumulation group (no Pool dependency).
    # fp8 variants carry a DoubleRow zero k-slot and -240 (e4m3 max is 448).
    id_d = nc.dram_tensor("ident", [P, P], BF16, kind="ExternalInput").ap()
    trim_d = nc.dram_tensor("trim", [P, 384], BF16, kind="ExternalInput").ap()
    id8_d = nc.dram_tensor("ident8", [P, 2, P], F8, kind="ExternalInput").ap()
    trim8_d = nc.dram_tensor("trim8", [P, 2, 384], F8, kind="ExternalInput").ap()
    # numer[h, d(64)+Z(1), tb, 256]: unnormalized attention + denominator row
    num_d = nc.dram_tensor("num", [NHC, HD + 1, T // 256, 256], F32,
                           kind="ExternalOutput").ap()

    with tile.TileContext(nc) as tc, ExitStack() as ctx:
        const = ctx.enter_context(tc.tile_pool(name="const", bufs=1))
        wpool = ctx.enter_context(tc.tile_pool(name="w", bufs=1))
        big = ctx.enter_context(tc.tile_pool(name="big", bufs=1))
        epool = ctx.enter_context(tc.tile_pool(name="e", bufs=8))
        opool = ctx.enter_context(tc.tile_pool(name="osb", bufs=4))
        ps_sc = ctx.enter_context(tc.tile_pool(name="ps_sc", bufs=3, space="PSUM"))
        ps_pv = ctx.enter_context(tc.tile_pool(name="ps_pv", bufs=1, space="PSUM"))

        dq = [nc.sync, nc.scalar, nc.gpsimd]

        # ---- load order tuned for ramp.  Few BIG DMAs: each HWDGE setup
        # costs ~630ns serially, so x^T loads are consolidated across kd. ----
        xT = big.tile([P, KD, T], BF16, tag="xT")
        xt_r = xt_d.rearrange("(kd p) t -> p kd t", p=P)

        LW = min(512, T)
        nc.gpsimd.dma_start(xT[:, 0:KD // 2, 0:LW], xt_r[:, 0:KD // 2, 0:LW])
        wq_sb = wpool.tile([P, KD, OC], BF16, tag="wq")
        nc.sync.dma_start(wq_sb[:, 0:KD // 2, :], wqt_d[:, 0:KD // 2, :])
        nc.scalar.dma_start(wq_sb[:, KD // 2:, :], wqt_d[:, KD // 2:, :])
        nc.gpsimd.dma_start(xT[:, KD // 2:, 0:LW], xt_r[:, KD // 2:, 0:LW])
        wk_sb = wpool.tile([P, KD, OC], BF16, tag="wk")
        nc.sync.dma_start(wk_sb[:], wkt_d[:])
        if T > LW:
            # second quarter next: the projection prefix pieces for chunk 0
            # block the in-order PE until these land
            nc.gpsimd.dma_start(xT[:, :, LW:2 * LW], xt_r[:, :, LW:2 * LW])
        wv_sb = wpool.tile([P, KD, OC], BF16, tag="wv")
        nc.scalar.dma_start(wv_sb[:], wvt_d[:])

        cst_sb = const.tile([P, 4 + NTT], F32)
        nc.sync.dma_start(cst_sb[:], cst_d[:])
        bq_sb = cst_sb[:, 0:2]
        bk_sb = cst_sb[:, 2:4]
        mask_sb = cst_sb[:, 4:4 + NTT]
        if SCORES_FP8:
            id8_sb = const.tile([P, 2, P], F8)
            nc.scalar.dma_start(id8_sb[:], id8_d[:])
            trim8_sb = const.tile([P, 2, 384], F8)
            nc.scalar.dma_start(trim8_sb[:], trim8_d[:])
        else:
            id_sb = const.tile([P, P], BF16)
            nc.scalar.dma_start(id_sb[:], id_d[:])
            trim_sb = const.tile([P, 384], BF16)
            nc.scalar.dma_start(trim_sb[:], trim_d[:])
        if T > 2 * LW:
            nc.gpsimd.dma_start(xT[:, :, 2 * LW:T], xt_r[:, :, 2 * LW:T])

        QK_DT = F8 if SCORES_FP8 else BF16
        # dims: [partition, o-tile, DoubleRow k-slot (1 = zeros), t]
        QT = big.tile([P, 2, 2, T], QK_DT, tag="QT")
        KT = big.tile([P, 2, 2, T], QK_DT, tag="KT")
        if SCORES_FP8:
            nc.gpsimd.memset(QT[:, :, 1, :], 0.0)
            nc.gpsimd.memset(KT[:, :, 1, :], 0.0)
        V = big.tile([P, NTT, NHC, HD + 1], BF16, tag="V")
        ones_sb = const.tile([P, 1], F32)
        nc.gpsimd.memset(ones_sb[:], 1.0)
        nc.vector.tensor_copy(
            V[:, :, :, HD:HD + 1].rearrange("p a b c -> p (a b c)"),
            ones_sb[:, 0:1].to_broadcast((P, NTT * NHC)),
        )

        def front_pieces(jp):
            """Front-end work for chunk jp as small thunks, interleaved into
            attention i-loops.  PSUM comes from the shared "sc" tag (one
            accumulation group per piece, bank-sequential)."""
            t0c = jp * CH

            def qk_piece(dst, w_sb, b_sb, ot, c0p, w):
                def go():
                    pr = ps_sc.tile([P, CH], F32, tag="sc",
                                    name=f"prj_{jp}_{c0p}_{ot}_{id(dst) % 97}")
                    for kd in range(KD):
                        nc.tensor.matmul(
                            pr[:, :w],
                            lhsT=w_sb[:, kd, ot * P:(ot + 1) * P],
                            rhs=xT[:, kd, t0c + c0p:t0c + c0p + w],
                            start=(kd == 0),
                            stop=(kd == KD - 1),
                        )
                    nc.vector.tensor_scalar_add(
                        dst[:, ot, 0, t0c + c0p:t0c + c0p + w],
                        pr[:, :w],
                        b_sb[:, ot:ot + 1],
                    )
                return go

            def v_piece(tt):
                def go():
                    pr = ps_sc.tile([P, CH], F32, tag="sc", name=f"prv_{tt}")
                    for kd in range(KD):
                        nc.tensor.matmul(
                            pr[:, :OC],
                            lhsT=xT[:, kd, tt * P:(tt + 1) * P],
                            rhs=wv_sb[:, kd, :],
                            start=(kd == 0),
                            stop=(kd == KD - 1),
                        )
                    nc.vector.tensor_copy(
                        V[:, tt, :, 0:HD],
                        pr[:, :OC].rearrange("p (h d) -> p h d", h=NHC)
                    )
                return go

            qs, ks, vs = [], [], []
            for c0p in range(0, CH, 512):
                w = min(512, CH - c0p)
                for ot in range(2):
                    qs.append(qk_piece(QT, wq_sb, bq_sb, ot, c0p, w))
                for ot in range(2):
                    ks.append(qk_piece(KT, wk_sb, bk_sb, ot, c0p, w))
                for tt in range((t0c + c0p) // P, (t0c + c0p + w) // P):
                    vs.append(v_piece(tt))
            return qs, ks, vs

        # ---- attention.  PV for step i is emitted LAG steps behind scores
        # so the in-order PE never waits on exp(i): while ACT computes
        # exp(i), the PE streams scores(i+1), scores(i+2).  With
        # deferred=<list>, PV is not emitted inline at all: thunks are
        # appended for later phases (pure-PE filler for exp-bound stretches);
        # the head's E tiles persist until those thunks consume them. ----
        def head_stream(j, h, deferred=None):
            ho, hb = divmod(h, 2)
            hbase = hb * 64
            imax = (CH * (j + 1) - 1) // P
            box = {}

            def get_pv():
                if "pv" not in box:
                    box["pv"] = ps_pv.tile([HD + 1, NQB, 256], F32, tag="pv",
                                           name=f"pv_{j}_{h}")
                    box["osb"] = opool.tile([HD + 1, CH], F32, tag="osb",
                                            name=f"osb_{j}_{h}")
                return box["pv"], box["osb"]

            Es = {}
            etag = "E0" if deferred is not None else "E"
            ebufs = 33 if deferred is not None else 8

            def kq_mm(ps, p0, s0, w, start, stop, i):
                if SCORES_FP8:
                    nc.tensor.matmul(
                        ps[:, p0:p0 + w],
                        lhsT=KT[hbase:hbase + 64, ho, :, i * P:(i + 1) * P],
                        rhs=QT[hbase:hbase + 64, ho, :,
                               j * CH + s0:j * CH + s0 + w],
                        start=start, stop=stop, perf_mode=DR,
                    )
                else:
                    nc.tensor.matmul(
                        ps[:, p0:p0 + w],
                        lhsT=KT[hbase:hbase + 64, ho, 0, i * P:(i + 1) * P],
                        rhs=QT[hbase:hbase + 64, ho, 0,
                               j * CH + s0:j * CH + s0 + w],
                        start=start, stop=stop,
                    )

            def emit_scores(g, gi):
                """Score+exp for a group of s-tiles.  Packed (len>1) groups
                share one PSUM tile and ONE exp; member data sits at
                anchor_a + col (anchored at its 256-block start, so every
                straddle hole is tri-blanked and exp-ed to ~0)."""
                ps = ps_sc.tile([P, CH], F32, tag="sc",
                                name=f"sc_{j}_{h}_{gi}")
                E = epool.tile([P, CH], BF16, tag=etag, bufs=ebufs,
                               name=f"E_{j}_{h}_{gi}")
                anchors = {}
                base = 0
                packed = len(g) > 1
                for a in g:
                    c0, diag, s0b = tile_geo(j, a)
                    anchor = (base - s0b) if packed else 0
                    anchors[a] = anchor
                    if diag:
                        # diag-containing 256-block: tri bias opens the
                        # group (-1e8/-240 on the masked region), KQ closes
                        off = P - (c0 - s0b)
                        w = min(256, CH - s0b)
                        if SCORES_FP8:
                            nc.tensor.matmul(
                                ps[:, anchor + s0b:anchor + s0b + w],
                                lhsT=id8_sb[:],
                                rhs=trim8_sb[:, :, off:off + w],
                                start=True, stop=False, perf_mode=DR,
                            )
                        else:
                            nc.tensor.matmul(
                                ps[:, anchor + s0b:anchor + s0b + w],
                                lhsT=id_sb[:],
                                rhs=trim_sb[:, off:off + w],
                                start=True, stop=False,
                            )
                        kq_mm(ps, anchor + s0b, s0b, w, False, True, a)
                        s0 = s0b + w
                    else:
                        s0 = s0b
                    while s0 < CH:
                        p0 = anchor + s0
                        w = min(512 if p0 % 512 == 0 else 256, CH - s0)
                        kq_mm(ps, p0, s0, w, True, True, a)
                        s0 += w
                    base += CH - s0b
                Es[gi] = (E, anchors)
                if packed:
                    nc.scalar.activation(
                        E[:, 0:base], ps[:, 0:base], AF.Exp,
                        scale=float(HD) ** -0.5,
                        bias=mask_sb[:, g[0]:g[0] + 1],
                    )
                else:
                    a = g[0]
                    c0, diag, s0b = tile_geo(j, a)
                    if c0 > s0b:
                        nc.gpsimd.memset(E[:, s0b:c0], 0.0)
                    nc.scalar.activation(
                        E[:, c0:CH], ps[:, c0:CH], AF.Exp,
                        scale=float(HD) ** -0.5, bias=mask_sb[:, a:a + 1],
                    )

            def emit_pv(g, gi):
                E, anchors = Es.pop(gi)
                for a in g:
                    emit_pv_one(a, E, anchors[a])

            def emit_pv_one(i, E, anchor):
                pv, osb = get_pv()
                c0, diag, s0b = tile_geo(j, i)
                if i == 0:
                    # one start=True matmul per PSUM BANK: a later start in
                    # the same bank wipes an open group's partial sum, so
                    # banks must be opened whole (2 qb blocks = 1 bank).
                    for qb0 in range(0, NQB, 2):
                        nb = min(2, NQB - qb0)
                        nc.tensor.matmul(
                            pv[:, qb0:qb0 + nb, :].rearrange(
                                "p a b -> p (a b)"),
                            lhsT=V[:, i, h, :],
                            rhs=E[:, anchor + qb0 * 256:
                                   anchor + (qb0 + nb) * 256],
                            start=True,
                            stop=False,
                        )
                    return
                qb = 0
                while qb < NQB:
                    s0 = qb * 256
                    if s0 + 256 <= s0b:
                        qb += 1
                        continue
                    lasti = min(imax, (256 * (qb + 1) + CH * j) // P - 1)
                    if (qb % 2 == 0 and qb + 1 < NQB and s0 >= s0b
                            and i < lasti):
                        # both blocks of this bank continue: one 512-wide
                        # accumulate
                        nc.tensor.matmul(
                            pv[:, qb:qb + 2, :].rearrange("p a b -> p (a b)"),
                            lhsT=V[:, i, h, :],
                            rhs=E[:, anchor + s0:anchor + s0 + 512],
                            start=False,
                            stop=False,
                            skip_group_check=True,
                        )
                        qb += 2
                        continue
                    nc.tensor.matmul(
                        pv[:, qb, :],
                        lhsT=V[:, i, h, :],
                        rhs=E[:, anchor + s0:anchor + s0 + 256],
                        start=False,
                        stop=(i == lasti),
                        skip_group_check=True,
                    )
                    if i == lasti:
                        nc.vector.tensor_copy(
                            osb[:, qb * 256:(qb + 1) * 256], pv[:, qb, :]
                        )
                        last_head = (j == NJ - 1 and h == NHC - 1)
                        if last_head or qb == NQB - 1:
                            b0 = qb if last_head else 0
                            tb0 = (CH * j) // 256
                            dq[(2 * h + qb) % 3].dma_start(
                                num_d[h, :, tb0 + b0:tb0 + qb + 1, :],
                                osb[:, b0 * 256:(qb + 1) * 256].rearrange(
                                    "p (tb ti) -> p tb ti", ti=256),
                            )
                    qb += 1

            plan = head_plan(j)
            ng = len(plan)
            if deferred is None:
                for k in range(ng + LAG):
                    if 0 <= k - LAG < ng:
                        emit_pv(plan[k - LAG], k - LAG)
                    if k < ng:
                        emit_scores(plan[k], k)
                    yield
            else:
                for k in range(ng):
                    emit_scores(plan[k], k)
                    yield
                for k in range(ng):
                    deferred.append(lambda k=k: emit_pv(plan[k], k))

        def tile_geo(j, i):
            s0_abs = i * P
            t0_abs = CH * j
            c0 = 0 if s0_abs < t0_abs else s0_abs - t0_abs
            diag = s0_abs >= t0_abs
            s0b = c0 - c0 % 256
            return c0, diag, s0b

        def head_plan(j):
            """s-tile processing groups: narrow diagonal tiles are packed
            (two-pointer: widest with narrowest that fits) into one
            scores-psum/exp when their block-anchored widths fit a single
            CH-wide tile (requires a uniform attention mask)."""
            imax = (CH * (j + 1) - 1) // P
            full = [i for i in range(imax + 1) if not tile_geo(j, i)[1]]
            diag = [i for i in range(imax + 1) if tile_geo(j, i)[1]]
            plan = [(i,) for i in full]
            if not PACKED:
                return plan + [(i,) for i in diag]
            wid = {i: CH - tile_geo(j, i)[2] for i in diag}
            lo, hi = 0, len(diag) - 1
            pairs = []
            while lo <= hi:
                if lo < hi and wid[diag[lo]] + wid[diag[hi]] <= CH:
                    pairs.append((diag[lo], diag[hi]))
                    lo += 1
                    hi -= 1
                else:
                    pairs.append((diag[lo],))
                    lo += 1
            return plan + pairs

        LAG = 3

        def n_steps(j, deferred=False):
            return len(head_plan(j)) + (0 if deferred else LAG)

        def run_streams(streams, pending):
            """Proportionally interleave step streams; pop one filler piece
            after each step (more if a backlog would remain)."""
            keyed = []
            for si, (gen, ln) in enumerate(streams):
                for k in range(ln):
                    keyed.append(((k + 0.5) / ln, si, gen))
            keyed.sort(key=lambda t: (t[0], t[1]))
            nleft = len(keyed)
            for _, _, gen in keyed:
                next(gen, None)
                nleft -= 1
                npop = 1 if pending else 0
                if len(pending) > nleft:
                    npop = max(npop, len(pending) - nleft)
                for _ in range(npop):
                    pending.pop(0)()
            for _, (gen, _ln) in enumerate(streams):
                for _ in gen:
                    pass

        f0q, f0k, f0v = front_pieces(0)
        if NJ == 1:
            for piece in f0q + f0k + f0v:
                piece()
            for h in range(NHC):
                run_streams([(head_stream(0, h), n_steps(0))], [])
        else:
            # Minimal serial prefix (what head (0,0) needs up front), then
            # every remaining projection piece dribbles into attention steps
            # as late as its first consumer allows, to fill the exp-bound
            # stretches with PE work.
            f1q, f1k, f1v = front_pieces(1)
            for piece in f0q:
                piece()
            f0k[0]()
            n0, n1 = n_steps(0), n_steps(1)
            run_streams([(head_stream(0, 0), n0)],
                        [f0v[0], f0v[1], f0k[2]] + f0v[2:8] + [f0k[1]])
            run_streams([(head_stream(0, 1), n0)], [f0k[3]] + f1q)
            run_streams([(head_stream(0, 2), n0)], [f1k[0], f1k[2]])
            run_streams([(head_stream(0, 3), n0)], [f1k[1], f1k[3]])
            run_streams([(head_stream(1, 0), n1)], list(f1v))
            for h in range(1, NHC):
                run_streams([(head_stream(1, h), n1)], [])

    nc.compile()
    return nc


def make_in_maps(hidden_states, attention_mask, Wq, bq, Aq, Bq, Wk, bk,
                 Wv, bv, Av, Bv):
    import ml_dtypes
    bf = ml_dtypes.bfloat16
    f32 = np.float32
    weff_q = np.asarray(Wq, f32) + f32(LORA_SCALE) * (
        np.asarray(Bq, f32) @ np.asarray(Aq, f32)
    )
    weff_v = np.asarray(Wv, f32) + f32(LORA_SCALE) * (
        np.asarray(Bv, f32) @ np.asarray(Av, f32)
    )
    Wk = np.asarray(Wk, f32)
    hs = np.asarray(hidden_states, f32)
    am = np.asarray(attention_mask, f32)
    bq = np.asarray(bq, f32)
    bk = np.asarray(bk, f32)
    T = hs.shape[1]
    KD = DM // P

    def warr(w_rows):
        # [OC, DM] row-slice -> lhsT layout [128, KD, OC] bf16
        return np.ascontiguousarray(
            w_rows.T.reshape(KD, P, OC).transpose(1, 0, 2).astype(bf)
        )

    ident = np.ascontiguousarray(np.eye(P, dtype=np.float32).astype(bf))
    trim = np.zeros((P, 384), np.float32)
    trim[:, 0:P] = -1e8
    for q in range(P):
        trim[q + 1:, P + q] = -1e8
    trimf = trim
    trim = np.ascontiguousarray(trim.astype(bf))
    f8 = ml_dtypes.float8_e4m3
    ident8 = np.zeros((P, 2, P), np.float32)
    ident8[:, 0, :] = np.eye(P, dtype=np.float32)
    ident8 = np.ascontiguousarray(ident8.astype(f8))
    trim8 = np.zeros((P, 2, 384), np.float32)
    trim8[:, 0, :] = np.where(trimf < 0, -240.0, 0.0)
    trim8 = np.ascontiguousarray(trim8.astype(f8))

    in_maps = []
    xt_by_batch = {}
    for c in range(8):
        b, g = divmod(c, 4)
        rows = slice(g * OC, (g + 1) * OC)
        if b not in xt_by_batch:
            xt_by_batch[b] = np.ascontiguousarray(hs[b].T.astype(bf))
        in_maps.append({
            "xt": xt_by_batch[b],
            "wqt": warr(weff_q[rows]),
            "wkt": warr(Wk[rows]),
            "wvt": warr(weff_v[rows]),
            "cst": np.ascontiguousarray(np.concatenate([
                bq[rows].reshape(2, P).T,
                bk[rows].reshape(2, P).T,
                am[b, 0, 0].reshape(T // P, P).T,
            ], axis=1)),
            "ident": ident,
            "trim": trim,
            "ident8": ident8,
            "trim8": trim8,
        })
    return in_maps


_NC_CACHE = {}


def kernel(hidden_states, attention_mask, Wq, bq, Aq, Bq, Wk, bk, Wv, bv,
           Av, Bv, _trace=False):
    T = np.asarray(hidden_states).shape[1]
    am = np.asarray(attention_mask)
    # packed exp groups share one per-partition bias column; only valid
    # when the attention mask is uniform
    packed = bool(np.all(am == am.flat[0]))
    key = (T, packed)
    if key not in _NC_CACHE:
        _NC_CACHE[key] = build_program(T, PACKED=packed)
    nc = _NC_CACHE[key]
    in_maps = make_in_maps(hidden_states, attention_mask, Wq, bq, Aq, Bq,
                           Wk, bk, Wv, bv, Av, Bv)
    res = None
    for attempt in range(3):
        try:
            res = run_bass_kernel_spmd(nc, in_maps, list(range(8)), trace=_trace)
            break
        except Exception:
            # transient NRT_EXEC_UNIT_UNRECOVERABLE device wedges recover on retry
            if attempt == 2:
                raise
            import time as _time
            _time.sleep(15)
    bv = np.asarray(bv, np.float32)
    out = np.empty((B, T, DM), np.float32)
    for c in range(8):
        b, g = divmod(c, 4)
        cols = slice(g * OC, (g + 1) * OC)
        num = res.results[c]["num"].reshape(NHC, HD + 1, T)  # [h, d|Z, t]
        o = num[:, :HD, :] / num[:, HD:HD + 1, :]            # [h, d, t]
        out[b, :, cols] = o.transpose(2, 0, 1).reshape(T, OC) + bv[cols][None, :]
    kernel.last_result = res
    return out


# revision 75
# speedup vs baseline: 1.0049x; 1.0042x over previous
"""Causal self-attention with LoRA (folded host-side), sharded over 8 NeuronCores.

Sharding: core c -> batch b = c//4, head-group g = c%4 (4 heads of 16).
Each core computes out[b, :, 256g:256g+256]; no collectives needed.

All matmuls run in bf16 (fp32 PSUM accumulation); host supplies x already
transposed and bf16-cast, so the device does no transposes at all:

  xT    [128p, kd(8), t]            DMA'd directly (host provides x^T bf16)
  Q^T/K^T [o(128p), ot(2), t] bf16  proj matmuls, lhsT=W^T tile, rhs=xT
  V_aug [s(128p), tt, h(4), 65]     proj matmuls, lhsT=xT tile, rhs=W^T;
                                    col 64 = ones (softmax denominator row)
  scores^T [s(128p), t-chunk] psum = K^T_h.T @ Q^T_h  (64-part contraction,
                                    head pairs at base partitions 0/64)
  E = exp(scores*0.125 + mask[s])   ACT -> bf16; causal: skip s-tiles above
                                    diag, memset sub-block zeros, gpsimd
                                    affine_select on diagonal blocks
  numer [65, qb(4), 256] psum      += V_aug_h.T @ E per 256-col block; row 64
                                    = Z[t]; DMA'd PSUM->DRAM when the block's
                                    accumulation stops.  Host does the final
                                    transpose + divide by Z.
"""

import numpy as np
from contextlib import ExitStack

import concourse.bass as bass
import concourse.tile as tile
from concourse import bacc, mybir
from concourse.bass_utils import run_bass_kernel_spmd

B, T_FULL, DM, H, R = 2, 2048, 1024, 16, 8
HD = 64
NHC = 4            # heads per core
OC = NHC * HD      # 256 out cols per core
LORA_SCALE = 16.0 / R
F32 = mybir.dt.float32
BF16 = mybir.dt.bfloat16
F8 = mybir.dt.float8e4
AF = mybir.ActivationFunctionType
ALU = mybir.AluOpType
DR = mybir.MatmulPerfMode.DoubleRow
P = 128
SCORES_FP8 = True  # QK^T in fp8e4m3 DoubleRow (zero-padded second k-slot)


def build_program(T=T_FULL, PACKED=True):
    KD = DM // P              # 8 contraction tiles
    NTT = T // P              # t 128-tiles
    CH = min(1024, T)         # attention t-chunk
    NJ = T // CH
    NQB = CH // 256           # 256-wide PV blocks per chunk

    nc = bacc.Bacc("TRN2", target_bir_lowering=False, debug=False)
    xt_d = nc.dram_tensor("xt", [DM, T], BF16, kind="ExternalInput").ap()
    wqt_d = nc.dram_tensor("wqt", [P, KD, OC], BF16, kind="ExternalInput").ap()
    wkt_d = nc.dram_tensor("wkt", [P, KD, OC], BF16, kind="ExternalInput").ap()
    wvt_d = nc.dram_tensor("wvt", [P, KD, OC], BF16, kind="ExternalInput").ap()
    # packed f32 consts: [0:2]=bq2, [2:4]=bk2, [4:4+NTT]=mask
    cst_d = nc.dram_tensor("cst", [P, 4 + NTT], F32, kind="ExternalInput").ap()
    # ident: identity; trim: [0:128]=-big, [128:256]=causal triangle
    # (-big where p > col-128), [256:384]=0.  Used to bias masked score
    # regions inside the scores PSUM accumulation group (no Pool dependency).
    # fp8 variants carry a DoubleRow zero k-slot and -240 (e4m3 max is 448).
    id_d = nc.dram_tensor("ident", [P, P], BF16, kind="ExternalInput").ap()
    trim_d = nc.dram_tensor("trim", [P, 384], BF16, kind="ExternalInput").ap()
    id8_d = nc.dram_tensor("ident8", [P, 2, P], F8, kind="ExternalInput").ap()
    trim8_d = nc.dram_tensor("trim8", [P, 2, 384], F8, kind="ExternalInput").ap()
    # numer[h, d(64)+Z(1), tb, 256]: unnormalized attention + denominator row
    num_d = nc.dram_tensor("num", [NHC, HD + 1, T // 256, 256], F32,
                           kind="ExternalOutput").ap()

    with tile.TileContext(nc) as tc, ExitStack() as ctx:
        const = ctx.enter_context(tc.tile_pool(name="const", bufs=1))
        wpool = ctx.enter_context(tc.tile_pool(name="w", bufs=1))
        big = ctx.enter_context(tc.tile_pool(name="big", bufs=1))
        epool = ctx.enter_context(tc.tile_pool(name="e", bufs=8))
        opool = ctx.enter_context(tc.tile_pool(name="osb", bufs=4))
        ps_sc = ctx.enter_context(tc.tile_pool(name="ps_sc", bufs=3, space="PSUM"))
        ps_pv = ctx.enter_context(tc.tile_pool(name="ps_pv", bufs=1, space="PSUM"))

        dq = [nc.sync, nc.scalar, nc.gpsimd]

        # ---- load order tuned for ramp.  Few BIG DMAs: each HWDGE setup
        # costs ~630ns serially, so x^T loads are consolidated across kd. ----
        xT = big.tile([P, KD, T], BF16, tag="xT")
        xt_r = xt_d.rearrange("(kd p) t -> p kd t", p=P)

        LW = min(512, T)
        nc.gpsimd.dma_start(xT[:, 0:KD // 2, 0:LW], xt_r[:, 0:KD // 2, 0:LW])
        wq_sb = wpool.tile([P, KD, OC], BF16, tag="wq")
        nc.sync.dma_start(wq_sb[:, 0:KD // 2, :], wqt_d[:, 0:KD // 2, :])
        nc.scalar.dma_start(wq_sb[:, KD // 2:, :], wqt_d[:, KD // 2:, :])
        nc.gpsimd.dma_start(xT[:, KD // 2:, 0:LW], xt_r[:, KD // 2:, 0:LW])
        wk_sb = wpool.tile([P, KD, OC], BF16, tag="wk")
        nc.sync.dma_start(wk_sb[:], wkt_d[:])
        if T > LW:
            # second quarter next: the projection prefix pieces for chunk 0
            # block the in-order PE until these land
            nc.gpsimd.dma_start(xT[:, :, LW:2 * LW], xt_r[:, :, LW:2 * LW])
        wv_sb = wpool.tile([P, KD, OC], BF16, tag="wv")
        nc.scalar.dma_start(wv_sb[:], wvt_d[:])

        cst_sb = const.tile([P, 4 + NTT], F32)
        nc.sync.dma_start(cst_sb[:], cst_d[:])
        bq_sb = cst_sb[:, 0:2]
        bk_sb = cst_sb[:, 2:4]
        mask_sb = cst_sb[:, 4:4 + NTT]
        if SCORES_FP8:
            id8_sb = const.tile([P, 2, P], F8)
            nc.scalar.dma_start(id8_sb[:], id8_d[:])
            trim8_sb = const.tile([P, 2, 384], F8)
            nc.scalar.dma_start(trim8_sb[:], trim8_d[:])
        else:
            id_sb = const.tile([P, P], BF16)
            nc.scalar.dma_start(id_sb[:], id_d[:])
            trim_sb = const.tile([P, 384], BF16)
            nc.scalar.dma_start(trim_sb[:], trim_d[:])
        if T > 2 * LW:
            nc.gpsimd.dma_start(xT[:, :, 2 * LW:T], xt_r[:, :, 2 * LW:T])

        QK_DT = F8 if SCORES_FP8 else BF16
        # dims: [partition, o-tile, DoubleRow k-slot (1 = zeros), t]
        QT = big.tile([P, 2, 2, T], QK_DT, tag="QT")
        KT = big.tile([P, 2, 2, T], QK_DT, tag="KT")
        if SCORES_FP8:
            nc.gpsimd.memset(QT[:, :, 1, :], 0.0)
            nc.gpsimd.memset(KT[:, :, 1, :], 0.0)
        V = big.tile([P, NTT, NHC, HD + 1], BF16, tag="V")
        ones_sb = const.tile([P, 1], F32)
        nc.gpsimd.memset(ones_sb[:], 1.0)
        nc.vector.tensor_copy(
            V[:, :, :, HD:HD + 1].rearrange("p a b c -> p (a b c)"),
            ones_sb[:, 0:1].to_broadcast((P, NTT * NHC)),
        )

        def front_pieces(jp):
            """Front-end work for chunk jp as small thunks, interleaved into
            attention i-loops.  PSUM comes from the shared "sc" tag (one
            accumulation group per piece, bank-sequential)."""
            t0c = jp * CH

            def qk_piece(dst, w_sb, b_sb, ot, c0p, w):
                def go():
                    pr = ps_sc.tile([P, CH], F32, tag="sc",
                                    name=f"prj_{jp}_{c0p}_{ot}_{id(dst) % 97}")
                    for kd in range(KD):
                        nc.tensor.matmul(
                            pr[:, :w],
                            lhsT=w_sb[:, kd, ot * P:(ot + 1) * P],
                            rhs=xT[:, kd, t0c + c0p:t0c + c0p + w],
                            start=(kd == 0),
                            stop=(kd == KD - 1),
                        )
                    nc.vector.tensor_scalar_add(
                        dst[:, ot, 0, t0c + c0p:t0c + c0p + w],
                        pr[:, :w],
                        b_sb[:, ot:ot + 1],
                    )
                return go

            def v_piece(tt):
                def go():
                    pr = ps_sc.tile([P, CH], F32, tag="sc", name=f"prv_{tt}")
                    for kd in range(KD):
                        nc.tensor.matmul(
                            pr[:, :OC],
                            lhsT=xT[:, kd, tt * P:(tt + 1) * P],
                            rhs=wv_sb[:, kd, :],
                            start=(kd == 0),
                            stop=(kd == KD - 1),
                        )
                    nc.vector.tensor_copy(
                        V[:, tt, :, 0:HD],
                        pr[:, :OC].rearrange("p (h d) -> p h d", h=NHC)
                    )
                return go

            qs, ks, vs = [], [], []
            for c0p in range(0, CH, 512):
                w = min(512, CH - c0p)
                for ot in range(2):
                    qs.append(qk_piece(QT, wq_sb, bq_sb, ot, c0p, w))
                for ot in range(2):
                    ks.append(qk_piece(KT, wk_sb, bk_sb, ot, c0p, w))
                for tt in range((t0c + c0p) // P, (t0c + c0p + w) // P):
                    vs.append(v_piece(tt))
            return qs, ks, vs

        # ---- attention.  PV for step i is emitted LAG steps behind scores
        # so the in-order PE never waits on exp(i): while ACT computes
        # exp(i), the PE streams scores(i+1), scores(i+2).  With
        # deferred=<list>, PV is not emitted inline at all: thunks are
        # appended for later phases (pure-PE filler for exp-bound stretches);
        # the head's E tiles persist until those thunks consume them. ----
        def head_stream(j, h, deferred=None):
            ho, hb = divmod(h, 2)
            hbase = hb * 64
            imax = (CH * (j + 1) - 1) // P
            box = {}

            def get_pv():
                if "pv" not in box:
                    box["pv"] = ps_pv.tile([HD + 1, NQB, 256], F32, tag="pv",
                                           name=f"pv_{j}_{h}")
                    box["osb"] = opool.tile([HD + 1, CH], F32, tag="osb",
                                            name=f"osb_{j}_{h}")
                return box["pv"], box["osb"]

            Es = {}
            etag = "E0" if deferred is not None else "E"
            ebufs = 33 if deferred is not None else 8

            def kq_mm(ps, p0, s0, w, start, stop, i):
                if SCORES_FP8:
                    nc.tensor.matmul(
                        ps[:, p0:p0 + w],
                        lhsT=KT[hbase:hbase + 64, ho, :, i * P:(i + 1) * P],
                        rhs=QT[hbase:hbase + 64, ho, :,
                               j * CH + s0:j * CH + s0 + w],
                        start=start, stop=stop, perf_mode=DR,
                    )
                else:
                    nc.tensor.matmul(
                        ps[:, p0:p0 + w],
                        lhsT=KT[hbase:hbase + 64, ho, 0, i * P:(i + 1) * P],
                        rhs=QT[hbase:hbase + 64, ho, 0,
                               j * CH + s0:j * CH + s0 + w],
                        start=start, stop=stop,
                    )

            def emit_scores(g, gi):
                """Score+exp for a group of s-tiles.  Packed (len>1) groups
                share one PSUM tile and ONE exp; member data sits at
                anchor_a + col (anchored at its 256-block start, so every
                straddle hole is tri-blanked and exp-ed to ~0)."""
                ps = ps_sc.tile([P, CH], F32, tag="sc",
                                name=f"sc_{j}_{h}_{gi}")
                E = epool.tile([P, CH], BF16, tag=etag, bufs=ebufs,
                               name=f"E_{j}_{h}_{gi}")
                anchors = {}
                base = 0
                packed = len(g) > 1
                for a in g:
                    c0, diag, s0b = tile_geo(j, a)
                    anchor = (base - s0b) if packed else 0
                    anchors[a] = anchor
                    if diag:
                        # diag-containing 256-block: tri bias opens the
                        # group (-1e8/-240 on the masked region), KQ closes
                        off = P - (c0 - s0b)
                        w = min(256, CH - s0b)
                        if SCORES_FP8:
                            nc.tensor.matmul(
                                ps[:, anchor + s0b:anchor + s0b + w],
                                lhsT=id8_sb[:],
                                rhs=trim8_sb[:, :, off:off + w],
                                start=True, stop=False, perf_mode=DR,
                            )
                        else:
                            nc.tensor.matmul(
                                ps[:, anchor + s0b:anchor + s0b + w],
                                lhsT=id_sb[:],
                                rhs=trim_sb[:, off:off + w],
                                start=True, stop=False,
                            )
                        kq_mm(ps, anchor + s0b, s0b, w, False, True, a)
                        s0 = s0b + w
                    else:
                        s0 = s0b
                    while s0 < CH:
                        p0 = anchor + s0
                        w = min(512 if p0 % 512 == 0 else 256, CH - s0)
                        kq_mm(ps, p0, s0, w, True, True, a)
                        s0 += w
                    base += CH - s0b
                Es[gi] = (E, anchors)
                if packed:
                    nc.scalar.activation(
                        E[:, 0:base], ps[:, 0:base], AF.Exp,
                        scale=float(HD) ** -0.5,
                        bias=mask_sb[:, g[0]:g[0] + 1],
                    )
                else:
                    a = g[0]
                    c0, diag, s0b = tile_geo(j, a)
                    if c0 > s0b:
                        nc.gpsimd.memset(E[:, s0b:c0], 0.0)
                    nc.scalar.activation(
                        E[:, c0:CH], ps[:, c0:CH], AF.Exp,
                        scale=float(HD) ** -0.5, bias=mask_sb[:, a:a + 1],
                    )

            def emit_pv(g, gi):
                E, anchors = Es.pop(gi)
                for a in g:
                    emit_pv_one(a, E, anchors[a])

            def emit_pv_one(i, E, anchor):
                pv, osb = get_pv()
                c0, diag, s0b = tile_geo(j, i)
                if i == 0:
                    # one start=True matmul per PSUM BANK: a later start in
                    # the same bank wipes an open group's partial sum, so
                    # banks must be opened whole (2 qb blocks = 1 bank).
                    for qb0 in range(0, NQB, 2):
                        nb = min(2, NQB - qb0)
                        nc.tensor.matmul(
                            pv[:, qb0:qb0 + nb, :].rearrange(
                                "p a b -> p (a b)"),
                            lhsT=V[:, i, h, :],
                            rhs=E[:, anchor + qb0 * 256:
                                   anchor + (qb0 + nb) * 256],
                            start=True,
                            stop=False,
                        )
                    return
                qb = 0
                while qb < NQB:
                    s0 = qb * 256
                    if s0 + 256 <= s0b:
                        qb += 1
                        continue
                    if (qb % 2 == 0 and qb + 1 < NQB and s0 >= s0b
                            and final_i[qb] != i and final_i[qb + 1] != i):
                        # both blocks of this bank continue: one 512-wide
                        # accumulate
                        nc.tensor.matmul(
                            pv[:, qb:qb + 2, :].rearrange("p a b -> p (a b)"),
                            lhsT=V[:, i, h, :],
                            rhs=E[:, anchor + s0:anchor + s0 + 512],
                            start=False,
                            stop=False,
                            skip_group_check=True,
                        )
                        qb += 2
                        continue
                    nc.tensor.matmul(
                        pv[:, qb, :],
                        lhsT=V[:, i, h, :],
                        rhs=E[:, anchor + s0:anchor + s0 + 256],
                        start=False,
                        stop=(final_i[qb] == i),
                        skip_group_check=True,
                    )
                    if final_i[qb] == i:
                        nc.vector.tensor_copy(
                            osb[:, qb * 256:(qb + 1) * 256], pv[:, qb, :]
                        )
                        last_head = (j == NJ - 1 and h == NHC - 1)
                        if last_head or qb == NQB - 1:
                            b0 = qb if last_head else 0
                            tb0 = (CH * j) // 256
                            dq[(2 * h + qb) % 3].dma_start(
                                num_d[h, :, tb0 + b0:tb0 + qb + 1, :],
                                osb[:, b0 * 256:(qb + 1) * 256].rearrange(
                                    "p (tb ti) -> p tb ti", ti=256),
                            )
                    qb += 1

            plan = head_plan(j)
            ng = len(plan)
            # plan-order-last contributing s-tile per q-block: that PV matmul
            # carries stop and triggers the output copy (accumulation order
            # is free, but the copy must follow the final contribution)
            pos = {}
            for k, g in enumerate(plan):
                for m, a in enumerate(g):
                    pos[a] = (k, m)
            final_i = {}
            for qb in range(NQB):
                contrib = [a for a in range(imax + 1)
                           if qb * 256 + 256 > tile_geo(j, a)[2]]
                final_i[qb] = max(contrib, key=lambda a: pos[a])
            if deferred is None:
                for k in range(ng + LAG):
                    if 0 <= k - LAG < ng:
                        emit_pv(plan[k - LAG], k - LAG)
                    if k < ng:
                        emit_scores(plan[k], k)
                    yield
            else:
                for k in range(ng):
                    emit_scores(plan[k], k)
                    yield
                for k in range(ng):
                    deferred.append(lambda k=k: emit_pv(plan[k], k))

        def tile_geo(j, i):
            s0_abs = i * P
            t0_abs = CH * j
            c0 = 0 if s0_abs < t0_abs else s0_abs - t0_abs
            diag = s0_abs >= t0_abs
            s0b = c0 - c0 % 256
            return c0, diag, s0b

        def head_plan(j):
            """s-tile processing groups: narrow diagonal tiles are packed
            (two-pointer: widest with narrowest that fits) into one
            scores-psum/exp when their block-anchored widths fit a single
            CH-wide tile (requires a uniform attention mask)."""
            imax = (CH * (j + 1) - 1) // P
            full = [i for i in range(imax + 1) if not tile_geo(j, i)[1]]
            diag = [i for i in range(imax + 1) if tile_geo(j, i)[1]]
            plan = [(i,) for i in full]
            if not PACKED:
                return plan + [(i,) for i in diag]
            wid = {i: CH - tile_geo(j, i)[2] for i in diag}
            lo, hi = 0, len(diag) - 1
            pairs = []
            while lo <= hi:
                if lo < hi and wid[diag[lo]] + wid[diag[hi]] <= CH:
                    pairs.append((diag[lo], diag[hi]))
                    lo += 1
                    hi -= 1
                else:
                    pairs.append((diag[lo],))
                    lo += 1
            return plan + pairs

        LAG = 3

        def n_steps(j, deferred=False):
            return len(head_plan(j)) + (0 if deferred else LAG)

        def run_streams(streams, pending):
            """Proportionally interleave step streams; pop one filler piece
            after each step (more if a backlog would remain)."""
            keyed = []
            for si, (gen, ln) in enumerate(streams):
                for k in range(ln):
                    keyed.append(((k + 0.5) / ln, si, gen))
            keyed.sort(key=lambda t: (t[0], t[1]))
            nleft = len(keyed)
            for _, _, gen in keyed:
                next(gen, None)
                nleft -= 1
                npop = 1 if pending else 0
                if len(pending) > nleft:
                    npop = max(npop, len(pending) - nleft)
                for _ in range(npop):
                    pending.pop(0)()
            for _, (gen, _ln) in enumerate(streams):
                for _ in gen:
                    pass

        f0q, f0k, f0v = front_pieces(0)
        if NJ == 1:
            for piece in f0q + f0k + f0v:
                piece()
            for h in range(NHC):
                run_streams([(head_stream(0, h), n_steps(0))], [])
        else:
            # Minimal serial prefix (what head (0,0) needs up front), then
            # every remaining projection piece dribbles into attention steps
            # as late as its first consumer allows, to fill the exp-bound
            # stretches with PE work.
            f1q, f1k, f1v = front_pieces(1)
            for piece in f0q:
                piece()
            f0k[0]()
            n0, n1 = n_steps(0), n_steps(1)
            run_streams([(head_stream(0, 0), n0)],
                        [f0v[0], f0v[1], f0k[2]] + f0v[2:8] + [f0k[1]])
            run_streams([(head_stream(0, 1), n0)], [f0k[3]] + f1q)
            run_streams([(head_stream(0, 2), n0)], [f1k[0], f1k[2]])
            run_streams([(head_stream(0, 3), n0)], [f1k[1], f1k[3]])
            run_streams([(head_stream(1, 0), n1)], list(f1v))
            for h in range(1, NHC):
                run_streams([(head_stream(1, h), n1)], [])

    nc.compile()
    return nc


def make_in_maps(hidden_states, attention_mask, Wq, bq, Aq, Bq, Wk, bk,
                 Wv, bv, Av, Bv):
    import ml_dtypes
    bf = ml_dtypes.bfloat16
    f32 = np.float32
    weff_q = np.asarray(Wq, f32) + f32(LORA_SCALE) * (
        np.asarray(Bq, f32) @ np.asarray(Aq, f32)
    )
    weff_v = np.asarray(Wv, f32) + f32(LORA_SCALE) * (
        np.asarray(Bv, f32) @ np.asarray(Av, f32)
    )
    Wk = np.asarray(Wk, f32)
    hs = np.asarray(hidden_states, f32)
    am = np.asarray(attention_mask, f32)
    bq = np.asarray(bq, f32)
    bk = np.asarray(bk, f32)
    T = hs.shape[1]
    KD = DM // P

    def warr(w_rows):
        # [OC, DM] row-slice -> lhsT layout [128, KD, OC] bf16
        return np.ascontiguousarray(
            w_rows.T.reshape(KD, P, OC).transpose(1, 0, 2).astype(bf)
        )

    ident = np.ascontiguousarray(np.eye(P, dtype=np.float32).astype(bf))
    trim = np.zeros((P, 384), np.float32)
    trim[:, 0:P] = -1e8
    for q in range(P):
        trim[q + 1:, P + q] = -1e8
    trimf = trim
    trim = np.ascontiguousarray(trim.astype(bf))
    f8 = ml_dtypes.float8_e4m3
    ident8 = np.zeros((P, 2, P), np.float32)
    ident8[:, 0, :] = np.eye(P, dtype=np.float32)
    ident8 = np.ascontiguousarray(ident8.astype(f8))
    trim8 = np.zeros((P, 2, 384), np.float32)
    trim8[:, 0, :] = np.where(trimf < 0, -240.0, 0.0)
    trim8 = np.ascontiguousarray(trim8.astype(f8))

    in_maps = []
    xt_by_batch = {}
    for c in range(8):
        b, g = divmod(c, 4)
        rows = slice(g * OC, (g + 1) * OC)
        if b not in xt_by_batch:
            xt_by_batch[b] = np.ascontiguousarray(hs[b].T.astype(bf))
        in_maps.append({
            "xt": xt_by_batch[b],
            "wqt": warr(weff_q[rows]),
            "wkt": warr(Wk[rows]),
            "wvt": warr(weff_v[rows]),
            "cst": np.ascontiguousarray(np.concatenate([
                bq[rows].reshape(2, P).T,
                bk[rows].reshape(2, P).T,
                am[b, 0, 0].reshape(T // P, P).T,
            ], axis=1)),
            "ident": ident,
            "trim": trim,
            "ident8": ident8,
            "trim8": trim8,
        })
    return in_maps


_NC_CACHE = {}


def kernel(hidden_states, attention_mask, Wq, bq, Aq, Bq, Wk, bk, Wv, bv,
           Av, Bv, _trace=False):
    T = np.asarray(hidden_states).shape[1]
    am = np.asarray(attention_mask)
    # packed exp groups share one per-partition bias column; only valid
    # when the attention mask is uniform
    packed = bool(np.all(am == am.flat[0]))
    key = (T, packed)
    if key not in _NC_CACHE:
        _NC_CACHE[key] = build_program(T, PACKED=packed)
    nc = _NC_CACHE[key]
    in_maps = make_in_maps(hidden_states, attention_mask, Wq, bq, Aq, Bq,
                           Wk, bk, Wv, bv, Av, Bv)
    res = None
    for attempt in range(3):
        try:
            res = run_bass_kernel_spmd(nc, in_maps, list(range(8)), trace=_trace)
            break
        except Exception:
            # transient NRT_EXEC_UNIT_UNRECOVERABLE device wedges recover on retry
            if attempt == 2:
                raise
            import time as _time
            _time.sleep(15)
    bv = np.asarray(bv, np.float32)
    out = np.empty((B, T, DM), np.float32)
    for c in range(8):
        b, g = divmod(c, 4)
        cols = slice(g * OC, (g + 1) * OC)
        num = res.results[c]["num"].reshape(NHC, HD + 1, T)  # [h, d|Z, t]
        o = num[:, :HD, :] / num[:, HD:HD + 1, :]            # [h, d, t]
        out[b, :, cols] = o.transpose(2, 0, 1).reshape(T, OC) + bv[cols][None, :]
    kernel.last_result = res
    return out
